# revision 49
# baseline (speedup 1.0000x reference)
"""Bending-energy loss kernel for Trainium2 (8 NeuronCores, Bass/Tile).

Input: ddf (4, 160, 160, 160, 3) fp32. Output: (4,) fp32 per-batch mean
bending energy.

Strategy ("G-harvest"): the energy is a fixed quadratic form in f — a
weighted sum of shifted-product sums  Sum f[p] * f[p + Delta]  over 13
offset families Delta = (dx, dy, dz), dx,|dy|,|dz| in {0,2,4}.  Per core
(batch x disjoint x-half) the volume is packed as [80 y-pairs, 2
y-parity, 80 x-blocks, 480 (c,z)] in fp8e4m3.  Five PE passes accumulate
shifted Gram matrices G[i,j] = Sum f[.., col i] * f_shift[.., col j]
with fp8 DoubleRow matmuls (0.5 cycles/row, both parities as the two
k-tiles); every device family is one diagonal of one G:  block offset =
dx, y-shift dy via a DMA-shifted copy xs1 (PE operands must start at
partition 0, so the shift is realized by re-DMAing the same DRAM buffer
at a +1 y-pair offset), column offset = dz (z is the fast column axis;
80-wide chunks keep pairs inside one c segment).  The host converts to
fp8, takes G diagonals with the exact stencil weights, computes the two
coefficient-2 second-neighbor families, and corrects thin window edges /
chunk-boundary gaps / x-half straddle pairs from raw data in fp64.
"""

import numpy as np
import ml_dtypes

B = 4
D = 160
C = 3
NCORES = 8

XIN = 80           # x planes per core (disjoint halves; straddle pairs on host)
NPAIR = 80         # y pairs
NCOL = 480         # (c, z) columns: col = c*160 + z
CH = 80            # chunk width (z pairs stay within one c segment half)
NCH = NCOL // CH   # 6 chunks
SLB = 9            # blocks per DMA slab

# The two coefficient-2 second-neighbor families are summed on the host
# from raw data (exact), alongside the other host corrections; the device
# computes every family with |stencil weight| >= 4 (>99.9% of the value).
HOST_FAMS = [(0, 4, 0), (4, 0, 0)]

# passes: (name, lhs_src, rhs_src, a (block offset), dy)
#   src: 0 = x (base), 1 = xs1 (y+2), 2 = xs2 (y+4)
PASSES = [
    ("p00", 0, 0, 0, 0),
    ("p20", 0, 0, 2, 0),
    ("p02", 0, 1, 0, 2),
    ("p22", 0, 1, 2, 2),
    ("p2m2", 1, 0, 2, -2),
]
if (4, 0, 0) not in HOST_FAMS:
    PASSES.insert(2, ("p40", 0, 0, 4, 0))
if (0, 4, 0) not in HOST_FAMS:
    PASSES.append(("p04", 0, 2, 0, 4))
NPASS = len(PASSES)
SRC_PARTS = {0: 80, 1: 79, 2: 78}
NSRC = 3 if (0, 4, 0) not in HOST_FAMS else 2
MAXA = max(p[3] for p in PASSES)

# family (dx,dy,dz) -> (pass index, diagonal offset)
_PASS_FAMS = {
    "p00": [((0, 0, 0), 0), ((0, 0, 2), 2), ((0, 0, 4), 4)],
    "p20": [((2, 0, 0), 0), ((2, 0, 2), 2), ((2, 0, -2), -2)],
    "p40": [((4, 0, 0), 0)],
    "p02": [((0, 2, 0), 0), ((0, 2, 2), 2), ((0, 2, -2), -2)],
    "p22": [((2, 2, 0), 0)],
    "p2m2": [((2, -2, 0), 0)],
    "p04": [((0, 4, 0), 0)],
}
FAM2PASS = {}
for _pi, _p in enumerate(PASSES):
    for _fam, _d in _PASS_FAMS[_p[0]]:
        FAM2PASS[_fam] = (_pi, _d)
FAMILIES = list(FAM2PASS) + HOST_FAMS

# fields: (energy weight, [(dx,dy,dz,coef)...])
FIELDS = [
    (1.0, [(-2, 0, 0, 1.0), (0, 0, 0, -2.0), (2, 0, 0, 1.0)]),
    (1.0, [(0, -2, 0, 1.0), (0, 0, 0, -2.0), (0, 2, 0, 1.0)]),
    (1.0, [(0, 0, -2, 1.0), (0, 0, 0, -2.0), (0, 0, 2, 1.0)]),
    (2.0, [(1, 1, 0, 1.0), (-1, 1, 0, -1.0), (1, -1, 0, -1.0), (-1, -1, 0, 1.0)]),
    (2.0, [(1, 0, 1, 1.0), (-1, 0, 1, -1.0), (1, 0, -1, -1.0), (-1, 0, -1, 1.0)]),
    (2.0, [(0, 1, 1, 1.0), (0, -1, 1, -1.0), (0, 1, -1, -1.0), (0, -1, -1, 1.0)]),
]

_cache = {}

# build-time tuning knobs (read once inside _build_program / _slabs)
_WARM = [0]
_PAD = [{}]
_PADMID = [{}]
_HEAD = [[3, 5, 8]]
_TAIL = [[6, 4]]


def _patch_tile(tile_mod, bass_rust, mybir):
    """Walrus on this toolchain accepts at most ONE sync wait per
    instruction: chain the TileContext exit drain as single-wait drains."""
    if getattr(tile_mod.TileContext, "_bending_patched", False):
        return

    def _drain_and_barrier_chunked(self, tick_clock, wait_clock):
        nc = self.nc
        gc = tick_clock.global_clock
        items = gc.items() if hasattr(gc, "items") else [(None, gc)]
        reqs = []
        for scope, vclock in items:
            for proc in range(len(vclock)):
                t = vclock[proc]
                if t > 0:
                    reqs.append((scope, proc, t))
        for scope, proc, t in reqs:
            sc = bass_rust.ScopedClock()
            sc.require_at_least(scope, proc, t)
            drain_inst = nc.sync.drain()
            wait_clock.add_sem_waits(drain_inst.ins, sc)
        if not reqs:
            nc.sync.drain()
        nc.all_engine_barrier()
        assert self.sems is not None
        popped = nc._tile_sem_poison_stack.pop()
        assert popped is self._sem_poison
        nc.clear_and_free_semaphores(list(self.sems.allocated().values()))
        nc.all_engine_barrier()

    tile_mod.TileContext._drain_and_barrier = _drain_and_barrier_chunked
    tile_mod.TileContext._bending_patched = True


_nop_counter = [0]


def _split_multi_waits(nc, mybir):
    """Split multi-wait instructions into single-wait NoOps + instruction."""
    for bb_name, bb_entry in list(nc.bb_map.items()):
        bb = bb_entry.bb if hasattr(bb_entry, "bb") else bb_entry
        insts = list(bb.instructions)
        new_insts = []
        changed = False
        for inst in insts:
            si = inst.sync_info
            if si is not None and si.on_wait is not None and len(si.on_wait) > 1:
                waits = list(si.on_wait)
                for w in waits[:-1]:
                    _nop_counter[0] += 1
                    nop = mybir.InstNoOp(
                        name=f"I-waitsplit-{_nop_counter[0]}",
                        engine=inst.engine,
                        ins=[],
                        outs=[],
                    )
                    nop.sync_info = mybir.SyncInfo(on_wait=[w], on_update=[])
                    new_insts.append(nop)
                inst.sync_info = mybir.SyncInfo(
                    on_wait=[waits[-1]], on_update=si.on_update
                )
                changed = True
            new_insts.append(inst)
        if changed:
            bb.instructions = new_insts


def _slabs():
    # graduated sizes: small slabs at both ends so PE starts early and the
    # final matmuls are not gated on a big trailing transfer
    head = list(_HEAD[0])
    tail = list(_TAIL[0])
    mid = XIN - sum(head) - sum(tail)
    sizes = list(head)
    while mid > SLB + 4:
        sizes.append(SLB)
        mid -= SLB
    sizes.append(mid)
    sizes += tail
    assert sum(sizes) == XIN and min(sizes) >= MAXA
    out = []
    b0 = 0
    for w in sizes:
        out.append((b0, b0 + w))
        b0 += w
    return out


def _build_program():
    import bass_rust
    import concourse.bass as bass
    import concourse.tile as tile
    import concourse.mybir as mybir

    _patch_tile(tile, bass_rust, mybir)

    fp8 = mybir.dt.float8e4
    f32 = mybir.dt.float32
    DR = mybir.MatmulPerfMode.DoubleRow

    slabs = _slabs()
    nslab = len(slabs)

    nc = bass.Bass()
    x_d = nc.declare_dram_parameter("x", [NPAIR, 2, XIN, NCOL], fp8, isOutput=False)
    g_d = nc.declare_dram_parameter("g", [CH, NPASS, CH], f32, isOutput=True)

    with tile.TileContext(nc) as tc:
        with (
            tc.tile_pool(name="xp", bufs=6) as xp,
            tc.tile_pool(name="x1p", bufs=6) as x1p,
            tc.tile_pool(name="x2p", bufs=3 if NSRC > 2 else 1) as x2p,
            tc.tile_pool(name="op", bufs=1) as op,
            tc.tile_pool(name="gacc", bufs=1, space="PSUM") as gaccp,
        ):
            G = gaccp.tile([CH, NPASS + 1, 128], f32, name="G")
            # PE warm-up: ~3us of dummy matmuls into a sacrificial PSUM
            # slot while the first DMA lands, so the p-state ramp finishes
            # before real work starts.  Input: small zeroed SBUF tile.
            wz = op.tile([80, 2, CH], fp8, name="wz")
            nc.vector.memzero(wz[:])
            wslot = NPASS
            wstate = {"n": 0}

            def warmup(n):
                for _ in range(n):
                    nc.tensor.matmul(
                        G[0:CH, wslot, 0:CH], wz[:], wz[:],
                        start=(wstate["n"] == 0), stop=False,
                        perf_mode=DR, skip_group_check=True,
                    )
                    wstate["n"] += 1

            warmup(_WARM[0])
            # start=True clears the WHOLE PSUM bank on hw: only the first
            # matmul landing in each 2KB bank may carry it (slot pi sits at
            # byte 512*pi -> bank pi//4); later matmuls accumulate, with
            # never-written regions reading as zero (has_written clear).
            bank_started = {}
            tiles = [[None] * nslab for _ in range(3)]

            def load_x(s):
                b0, b1 = slabs[s]
                w = b1 - b0
                t0 = xp.tile([80, 2, w, NCOL], fp8, name="tx")
                nc.gpsimd.dma_start(t0[:], x_d[0:80, :, b0:b1, :])
                tiles[0][s] = t0

            def load_s1(s):
                b0, b1 = slabs[s]
                w = b1 - b0
                t1 = x1p.tile([79, 2, w, NCOL], fp8, name="t1")
                nc.gpsimd.dma_start(t1[:], x_d[1:80, :, b0:b1, :])
                tiles[1][s] = t1
                if NSRC > 2:
                    t2 = x2p.tile([78, 2, w, NCOL], fp8, name="t2")
                    nc.gpsimd.dma_start(t2[:], x_d[2:80, :, b0:b1, :])
                    tiles[2][s] = t2

            def view(src, k):
                for s, (b0, b1) in enumerate(slabs):
                    if b0 <= k < b1:
                        return tiles[src][s], k - b0
                raise AssertionError(k)

            frontier = [0] * NPASS
            gc_ = op.tile([CH, NPASS, CH], f32, name="gc")

            def compute(s):
                # emit, per pass, every block whose lhs AND rhs slabs are
                # loaded by slab s (cross-boundary blocks lag one slab)
                b0, b1 = slabs[s]
                last = s + 1 >= nslab
                for pi, (_, ls, rs, a, _dy) in enumerate(PASSES):
                    P = min(SRC_PARTS[ls], SRC_PARTS[rs])
                    if last:
                        hi = XIN - a
                    elif ls == 1 or rs == 1:
                        # xs1-dependent passes lag one slab (their transfer
                        # stream lags the x stream in the DMA queue)
                        hi = min(XIN - a, (slabs[s - 1][1] if s > 0 else 0) - a)
                    else:
                        hi = min(XIN - a, b1 - a)
                    hi = max(hi, frontier[pi])
                    for k in range(frontier[pi], hi):
                        lt, lk = view(ls, k)
                        rt, rk = view(rs, k + a)
                        for q in range(NCH):
                            c0 = q * CH
                            bank = (pi * 512) // 2048
                            st = not bank_started.get(bank, False)
                            nc.tensor.matmul(
                                G[0:CH, pi, 0:CH],
                                lt[0:P, :, lk, c0 : c0 + CH],
                                rt[0:P, :, rk, c0 : c0 + CH],
                                start=st, stop=False,
                                perf_mode=DR, skip_group_check=True,
                            )
                            bank_started[bank] = True
                    frontier[pi] = hi
                    if pi == 1:
                        warmup(_PADMID[0].get(s, 0))
                    if last and pi == NPASS - 1:
                        nc.scalar.activation(
                            gc_[:], G[0:CH, 0:NPASS, 0:CH],
                            mybir.ActivationFunctionType.Copy,
                        )
                        nc.gpsimd.dma_start(g_d[:], gc_[:])

            load_x(0)
            pad = dict(_PAD[0])
            for s in range(nslab):
                if s + 1 < nslab:
                    load_x(s + 1)
                load_s1(s)
                compute(s)
                warmup(pad.get(s, 0))

    _split_multi_waits(nc, mybir)
    return nc


def _pack_inputs(ddf):
    """Per-core fp8-packed volumes [80, 2, 82, 480]."""
    in_maps = []
    packed_b = {}
    for core in range(NCORES):
        b, h = core // 2, core % 2
        if b not in packed_b:
            # (x,y,z,c) -> (y, x, c, z) -> (80, 2, 160, 480)
            v = ddf[b].transpose(1, 0, 3, 2).reshape(NPAIR, 2, D, NCOL)
            packed_b[b] = v.astype(ml_dtypes.float8_e4m3fn)
        x0 = h * XIN
        xq = np.ascontiguousarray(packed_b[b][:, :, x0 : x0 + XIN, :])
        in_maps.append({"x": xq})
    return in_maps


# ---------- host assembly ----------

def _maxbox(delta):
    dx, dy, dz = delta
    return (
        (max(0, -dx), D - max(0, dx)),
        (max(0, -dy), D - max(0, dy)),
        (max(0, -dz), D - max(0, dz)),
    )


def _ups(f, delta, xr, yr, zr):
    """Sum over base ranges of f[p]*f[p+delta], all c. f: (160,160,160,3)."""
    dx, dy, dz = delta
    x0, x1 = xr
    y0, y1 = yr
    z0, z1 = zr
    if x1 <= x0 or y1 <= y0 or z1 <= z0:
        return 0.0
    A = f[x0:x1, y0:y1, z0:z1, :].astype(np.float64)
    Bv = f[x0 + dx : x1 + dx, y0 + dy : y1 + dy, z0 + dz : z1 + dz, :].astype(
        np.float64
    )
    return float(np.sum(A * Bv))


def _canon_cols(f, delta, xr, yr):
    """Product sum over the device's canonical z set (pairs within the two
    80-wide chunks per c segment)."""
    dz = delta[2]
    bz = _maxbox(delta)[2]
    s = 0.0
    for zb in (0, CH):
        if dz >= 0:
            zr = (zb, zb + CH - dz)
        else:
            zr = (zb - dz, zb + CH)
        zr = (max(zr[0], bz[0]), min(zr[1], bz[1]))
        s += _ups(f, delta, xr, yr, zr)
    return s


def _gapsum(f, delta, xr, yr):
    """Pairs straddling the within-c chunk boundary (z = 80)."""
    dz = delta[2]
    bz = _maxbox(delta)[2]
    s = 0.0
    if dz > 0:
        zrs = [(CH - dz, CH)]
    elif dz < 0:
        zrs = [(CH, CH - dz)]
    else:
        return 0.0
    for (g0, g1) in zrs:
        for zb in (0, CH):
            zr = (max(zb + g0, bz[0]), min(zb + g1, bz[1]))
            s += _ups(f, delta, zr=(zr[0], zr[1]), xr=xr, yr=yr)
    return s


def _assemble_batch(gs, f_raw, f_q):
    """gs: 2 per-core G arrays [CH, NPASS, CH].  Unnormalized energy sum."""
    T = {}
    for fam, (pi, dd) in FAM2PASS.items():
        t = 0.0
        for h in range(2):
            g = gs[h][:, pi, :].astype(np.float64)
            t += np.trace(g, offset=dd)
        T[fam] = t

    U = {}
    for fam in FAMILIES:
        bx, by, bz = _maxbox(fam)
        if fam in HOST_FAMS:
            U[fam] = _ups(f_raw, fam, bx, by, bz)
            continue
        a = fam[0]
        u = T[fam]
        # cores cover x-pair bases [0, 80-a) and [80, 160-a): add the
        # straddle pairs [80-a, 80) (full z) and the chunk-boundary z gaps
        # over the two device x segments
        if a > 0:
            u += _ups(f_q, fam, (XIN - a, XIN), by, bz)
        u += _gapsum(f_q, fam, (0, XIN - a), by)
        u += _gapsum(f_q, fam, (XIN, D - a), by)
        U[fam] = u

    total = 0.0
    for wf, taps in FIELDS:
        for i, (tx, ty, tz, ct) in enumerate(taps):
            for j, (ux, uy, uz, cu) in enumerate(taps):
                if j < i:
                    continue
                mult = (1.0 if i == j else 2.0) * wf * ct * cu
                delta = (ux - tx, uy - ty, uz - tz)
                base = (tx, ty, tz)
                neg = False
                for dcomp in delta:
                    if dcomp != 0:
                        neg = dcomp < 0
                        break
                if neg:
                    delta = (-delta[0], -delta[1], -delta[2])
                    base = (ux, uy, uz)
                bt = tuple((2 + bb, D - 2 + bb) for bb in base)
                mbx, mby, mbz = _maxbox(delta)
                btx, bty, btz = bt
                ps = U[delta]
                ps -= _ups(f_raw, delta, (mbx[0], btx[0]), mby, mbz)
                ps -= _ups(f_raw, delta, (btx[1], mbx[1]), mby, mbz)
                ps -= _ups(f_raw, delta, btx, (mby[0], bty[0]), mbz)
                ps -= _ups(f_raw, delta, btx, (bty[1], mby[1]), mbz)
                ps -= _ups(f_raw, delta, btx, bty, (mbz[0], btz[0]))
                ps -= _ups(f_raw, delta, btx, bty, (btz[1], mbz[1]))
                total += mult * ps
    return total


def kernel(ddf):
    ddf = np.asarray(ddf, dtype=np.float32)
    if "prog" not in _cache:
        _cache["prog"] = (_build_program(), NPASS)
    nc, _ = _cache["prog"]

    from concourse.bass_utils import run_bass_kernel_spmd

    in_maps = _pack_inputs(ddf)
    res = run_bass_kernel_spmd(nc, in_maps, list(range(NCORES)))

    out = np.zeros(B, np.float64)
    for b in range(B):
        f_raw = ddf[b]
        f_q = ddf[b].astype(ml_dtypes.float8_e4m3fn).astype(np.float32)
        gs = [np.asarray(res.results[2 * b + h]["g"]) for h in range(2)]
        out[b] = _assemble_batch(gs, f_raw, f_q)
    out /= 16.0 * (156 ** 3) * 3
    return out.astype(np.float32)
